# revision 25
# baseline (speedup 1.0000x reference)
"""Multi-head causal self-attention (RoPE) on 8 TRN2 NeuronCores.

Strategy (tensor-parallel over heads, per the sharding hint):
  - 16 heads / 8 cores -> 2 heads per core. Each core processes ALL 4
    batches for its 2 heads:
      qkv slice -> RoPE -> causal softmax(q k^T) v -> partial out-proj
    and writes a full-shape partial y (row-parallel w_proj). The host
    sums the 8 partials and adds b_proj.
  - All matmul operands are fp16 (PSUM accumulation stays fp32): the PE
    streams 1 row/cycle at any moving-dim size, DMA + SBUF traffic
    halve, and accuracy (~1e-3) is far inside the 2e-2 gate.
  - x is sent pre-transposed (x^T, [C, T] per batch) so the contraction
    dim C lands on SBUF partitions with no on-device transposes.
  - Attention runs in the "S^T" layout (k-tokens on partitions,
    q-tokens on the free dim), q-chunks of 256:
      S^T pair  = two matmuls (k-tiles 2p, 2p+1) into ONE psum bank
                  (start on the first zeroes the bank, stop on the 2nd)
      P^T pair  = exp(S^T * 1/sqrt(D))  (one ACT op per bank; no
                  max-subtraction: |scores| <~ 6 so exp is safe)
      denom     = DVE accumulates P columns, then ONE gpsimd
                  partition_all_reduce gives the broadcast k-sum
                  (no PE cycles and no separate broadcast)
      out^T    += v[ktile]^T P^T per k-tile, normalized at eviction by
                  reciprocal_approx_fast(denom).
    attn_out^T ([d, tok]) is directly the lhsT of the out-projection.
  - RoPE: the head dim d sits on partitions; rotate_half needs rows
    d <-> d+-64. We permute the d index on the host (within each head's
    128 columns of w_qkv + the cos/sin tables) so that rotation partners
    sit 16 apart inside the same 32-partition quadrant, which a single
    DVE stream_shuffle implements. Scores are invariant to the (shared)
    q/k permutation. The psum tile is first down-cast to fp16 by the
    ACT engine so all four RoPE DVE ops run on 16-bit data.
  - Per-batch q/k/v/attn-out stores are double-buffered (b%2) so batch
    b+1's projection overlaps batch b's attention/out-projection.
"""

import os
from contextlib import ExitStack

import numpy as np

import concourse.bacc as bacc
import concourse.bass as bass
import concourse.mybir as mybir
import concourse.tile as tile
from concourse.bass import ds, ts
from concourse.bass_isa import ReduceOp

B = 4
T = 2048
C = 2048
H = 16
D = 128
NCORES = 8
HPC = H // NCORES  # heads per core = 2
KC = C // 128  # 16 contraction tiles
TT = T // 128  # 16 token tiles
ACH = 256  # stage-A token chunk
NACH = T // ACH
QCH = 256  # stage-B q chunk
NQCH = T // QCH
INV_SQRT_D = float(1.0 / np.sqrt(np.float32(D)))

F32 = mybir.dt.float32
F16 = mybir.dt.float16

# d-permutation: quadrant s holds original d = s*16..s*16+15 (rows 0-15)
# and d+64 partners (rows 16-31); swap = stream_shuffle by +-16.
PERM = np.concatenate(
    [np.concatenate([np.arange(s * 16, s * 16 + 16), 64 + np.arange(s * 16, s * 16 + 16)]) for s in range(4)]
).astype(np.int64)
SWAP_MASK = [(i + 16) % 32 for i in range(32)]


def _mm(nc, out, lhsT, rhs, **kw):
    nc.tensor.matmul(out, lhsT, rhs, **kw)


def build_program():
    nc = bacc.Bacc("TRN2", target_bir_lowering=False, debug=False, num_devices=NCORES)

    xt = nc.dram_tensor("xt", [B, KC, 128, T], F16, kind="ExternalInput").ap()
    wqk = nc.dram_tensor("wqk", [KC, 128, 4 * 128], F16, kind="ExternalInput").ap()
    wv = nc.dram_tensor("wv", [KC, 128, 2 * 128], F16, kind="ExternalInput").ap()
    wproj = nc.dram_tensor("wproj", [HPC, 128, C], F16, kind="ExternalInput").ap()
    cos_in = nc.dram_tensor("cos_t", [128, T], F16, kind="ExternalInput").ap()
    sin_in = nc.dram_tensor("sin_t", [128, T], F16, kind="ExternalInput").ap()
    masks = nc.dram_tensor("masks", [128, 2 * QCH], F16, kind="ExternalInput").ap()
    ones_c = nc.dram_tensor("ones_c", [128, 1], F16, kind="ExternalInput").ap()
    ones_r = nc.dram_tensor("ones_r", [1, 128], F16, kind="ExternalInput").ap()
    y = nc.dram_tensor("y", [B, TT, 128, C], F16, kind="ExternalOutput").ap()

    with TileKernel(nc) as tk:
        tk.build(xt, wqk, wv, wproj, cos_in, sin_in, masks, ones_c, ones_r, y)
    nc.compile()
    return nc


class TileKernel:
    def __init__(self, nc):
        self.nc = nc
        self.stack = ExitStack()

    def __enter__(self):
        self.tc = self.stack.enter_context(tile.TileContext(self.nc))
        return self

    def __exit__(self, *exc):
        return self.stack.__exit__(*exc)

    def build(self, xt, wqk, wv, wproj, cos_in, sin_in, masks, ones_c, ones_r, y):
        nc, tc = self.nc, self.tc
        ctx = self.stack

        consts = ctx.enter_context(tc.tile_pool(name="consts", bufs=1))
        store = ctx.enter_context(tc.tile_pool(name="store", bufs=1))
        xtp = ctx.enter_context(tc.tile_pool(name="xtp", bufs=3))
        ropep = ctx.enter_context(tc.tile_pool(name="ropep", bufs=3))
        pp = ctx.enter_context(tc.tile_pool(name="pp", bufs=6))
        dp = ctx.enter_context(tc.tile_pool(name="dp", bufs=3))
        evp = ctx.enter_context(tc.tile_pool(name="evp", bufs=3))

        # prefetch the first two x chunks ahead of the bulk weight DMAs so
        # the first matmuls aren't stuck behind 3MB on the sync queue
        pf = {}
        for c in range(2):
            xt_pf = xtp.tile([128, KC, ACH], F16, tag="xt", name=f"xt_pf{c}")
            if c == 0:
                # sub-split so the kc=0 slice lands first and matmuls start early
                for k0 in range(0, KC, 4):
                    nc.sync.dma_start(out=xt_pf[:, ds(k0, 4), :],
                                      in_=xt[0, ds(k0, 4), :, ds(0, ACH)].transpose([1, 0, 2]))
            else:
                nc.sync.dma_start(out=xt_pf, in_=xt[0, :, :, ds(c * ACH, ACH)].transpose([1, 0, 2]))
            pf[c] = xt_pf

        # ---- persistent weights / tables ----
        wqk_sb = consts.tile([128, KC, 512], F16)
        wv_sb = consts.tile([128, KC, 256], F16)
        wproj_sb = consts.tile([128, HPC, C], F16)
        cos_sb = consts.tile([128, T], F16)
        sin_sb = consts.tile([128, T], F16)
        mask_sb = consts.tile([128, 2 * QCH], F16)
        ones_col = consts.tile([128, 1], F16)
        ones_row = consts.tile([1, 128], F16)
        nc.sync.dma_start(out=ones_col, in_=ones_c)
        nc.sync.dma_start(out=ones_row, in_=ones_r)
        # weights in kc-chunks so the first chunk's matmuls unblock before
        # the whole 3MB of weights has landed
        for k0 in range(0, KC, 4):
            nc.sync.dma_start(out=wqk_sb[:, ds(k0, 4), :], in_=wqk[ds(k0, 4)].transpose([1, 0, 2]))
            nc.sync.dma_start(out=wv_sb[:, ds(k0, 4), :], in_=wv[ds(k0, 4)].transpose([1, 0, 2]))
        nc.sync.dma_start(out=cos_sb, in_=cos_in)
        nc.sync.dma_start(out=sin_sb, in_=sin_in)

        # ---- per-batch stores, double-buffered over batches ----
        q_t = [[store.tile([128, T], F16, name=f"q_t{s}_{h}") for h in range(HPC)] for s in range(2)]
        k_t = [[store.tile([128, T], F16, name=f"k_t{s}_{h}") for h in range(HPC)] for s in range(2)]
        v_sb = [store.tile([128, TT, 256], F16, name=f"v_sb{s}") for s in range(2)]
        ao_t = [[store.tile([128, T], F16, name=f"ao_t{s}_{h}") for h in range(HPC)] for s in range(2)]

        for b in range(B):
            s = b % 2
            self._stage_a(b, s, xt, wqk_sb, wv_sb, cos_sb, sin_sb, q_t, k_t, v_sb, xtp, ropep, pf if b == 0 else None)
            if b == 0:
                # stage-B/C weights load behind batch-0 qkv so the first
                # matmuls aren't queued behind not-yet-needed DMAs
                nc.sync.dma_start(out=mask_sb, in_=masks)
                nc.sync.dma_start(out=wproj_sb, in_=wproj.transpose([1, 0, 2]))
            for h in range(HPC):
                self._stage_b(s, h, q_t, k_t, v_sb, ao_t, mask_sb, ones_col, ones_row, pp, dp)
            self._stage_c(b, s, ao_t, wproj_sb, y, evp)

    # qkv projection + RoPE for batch b
    def _stage_a(self, b, s, xt, wqk_sb, wv_sb, cos_sb, sin_sb, q_t, k_t, v_sb, xtp, ropep, pf=None):
        nc, tc = self.nc, self.tc
        # Two 256-wide accumulators share each PSUM bank; bufs=2 double-buffers
        # chunks so the PE never waits on the RoPE/v evictions.
        with tc.tile_pool(name=f"psA{b}", bufs=2, space="PSUM") as psA:
            for c in range(NACH):
                seg = ds(c * ACH, ACH)
                if pf and c in pf:
                    xt_all = pf.pop(c)
                else:
                    # one batched DMA per chunk: [KC, 128, ACH] -> [128, KC, ACH]
                    xt_all = xtp.tile([128, KC, ACH], F16, tag="xt", name=f"xt_{b}_{c}")
                    nc.sync.dma_start(out=xt_all, in_=xt[b, :, :, seg].transpose([1, 0, 2]))
                xt_tiles = [xt_all[:, kc, :] for kc in range(KC)]
                ps_b = [psA.tile([128, 2 * ACH], F32, tag=f"qkb{p}", name=f"psqkb{p}") for p in range(2)]
                ps_vb = psA.tile([128, 2 * 256], F32, tag="vb", name="psvb")
                ps_qk = [ps_b[m // 2][:, ds((m % 2) * ACH, ACH)] for m in range(4)]
                ps_v = [ps_vb[:, ds(t * 256, 256)] for t in range(ACH // 128)]
                # the two accumulators in one bank form a single group:
                # start zeroes the whole 2KB region, so only the first
                # matmul into a bank starts and only the last stops.
                for kc in range(KC):
                    for m in range(4):
                        _mm(nc, ps_qk[m], wqk_sb[:, kc, ds(m * 128, 128)], xt_tiles[kc],
                            start=(kc == 0 and m % 2 == 0), stop=(kc == KC - 1 and m % 2 == 1))
                    for t in range(ACH // 128):
                        _mm(nc, ps_v[t], xt_tiles[kc][:, ds(t * 128, 128)], wv_sb[:, kc, :],
                            start=(kc == 0 and t == 0), stop=(kc == KC - 1 and t == 1))
                # RoPE eviction: m -> (q/k, head). ACT downcasts the psum
                # tile to fp16 first so the DVE ops run on 16-bit data.
                for m in range(4):
                    h = m % 2
                    dst = (q_t if m < 2 else k_t)[s][h]
                    qf = ropep.tile([128, ACH], F16, tag="qf", name="qf")
                    sw = ropep.tile([128, ACH], F16, tag="sw", name="sw")
                    t1 = ropep.tile([128, ACH], F16, tag="t1", name="t1")
                    nc.scalar.copy(qf, ps_qk[m])
                    nc.vector.stream_shuffle(sw, qf, mask=SWAP_MASK)
                    nc.vector.tensor_mul(t1, qf, cos_sb[:, seg])
                    nc.vector.tensor_mul(sw, sw, sin_sb[:, seg])
                    nc.vector.tensor_add(dst[:, seg], t1, sw)
                for t in range(ACH // 128):
                    nc.scalar.copy(v_sb[s][:, c * (ACH // 128) + t, :], ps_v[t])

    # causal attention for head h (current batch): fills ao_t[s][h]
    def _stage_b(self, s, h, q_t, k_t, v_sb, ao_t, mask_sb, ones_col, ones_row, pp, dp):
        nc, tc = self.nc, self.tc
        with (
            tc.tile_pool(name=f"psS{h}", bufs=3, space="PSUM") as psS,
            tc.tile_pool(name=f"psO{h}", bufs=3, space="PSUM") as psO,
            tc.tile_pool(name=f"psR{h}", bufs=2, space="PSUM") as psR,
        ):
            for jc in range(NQCH):
                qseg = ds(jc * QCH, QCH)
                npairs = jc + 1  # k-tile pairs 2p, 2p+1 with 2p+1 <= 2jc+1
                # O accumulator shares its psum bank with the denominator
                # row: the den-sum matmul is the group's closing member.
                ps_ob = psO.tile([128, 2 * QCH], F32, tag="o", name="ps_ob")
                ps_o = ps_ob[:, ds(0, QCH)]
                # two independent denominator accumulators (left/right pair
                # halves) halve the serial DVE chain that paces the pipeline
                den_l = dp.tile([128, QCH], F16, tag="denl", name="den_l")
                den_r = dp.tile([128, QCH], F16, tag="denr", name="den_r")
                den = dp.tile([128, QCH], F16, tag="den", name="den")
                # software pipeline: the O matmuls consume ptile LAG pairs
                # behind the S-matmul/exp/mask producers so the PE never
                # waits on ACT/DVE.
                LAG = 2
                ptiles = {}
                for i in range(npairs + LAG):
                    if i < npairs:
                        diag = i == npairs - 1
                        # the diagonal pair's 2nd k-tile only attends to the
                        # 2nd q-half: its first 128 q columns are skipped
                        w = QCH + 128 if diag else 2 * QCH
                        ps_s = psS.tile([128, 2 * QCH], F32, tag="s", name="ps_s")
                        _mm(nc, ps_s[:, ds(0, QCH)], k_t[s][h][:, ds(2 * i * 128, 128)], q_t[s][h][:, qseg],
                            start=True, stop=False)
                        _mm(nc, ps_s[:, ds(QCH, w - QCH)], k_t[s][h][:, ds((2 * i + 1) * 128, 128)],
                            q_t[s][h][:, ds(jc * QCH + (2 * QCH - w), w - QCH)],
                            start=False, stop=True)
                        ptile = pp.tile([128, 2 * QCH], F16, tag="pt", name="ptile")
                        nc.scalar.activation(ptile[:, ds(0, w)], ps_s[:, ds(0, w)],
                                             mybir.ActivationFunctionType.Exp, scale=INV_SQRT_D)
                        if diag:
                            nc.vector.tensor_mul(ptile[:, ds(0, w)], ptile[:, ds(0, w)], mask_sb[:, ds(0, w)])
                        if i == 0 and npairs == 1:
                            nc.vector.tensor_copy(den, ptile[:, ds(0, QCH)])
                            nc.vector.tensor_add(den[:, ds(128, 128)], den[:, ds(128, 128)],
                                                 ptile[:, ds(QCH, 128)])
                        elif i == 0:
                            nc.vector.tensor_copy(den_l, ptile[:, ds(0, QCH)])
                            nc.vector.tensor_copy(den_r, ptile[:, ds(QCH, QCH)])
                        elif diag:
                            nc.vector.tensor_add(den_l, den_l, ptile[:, ds(0, QCH)])
                            nc.vector.tensor_add(den_r[:, ds(128, 128)], den_r[:, ds(128, 128)],
                                                 ptile[:, ds(QCH, 128)])
                        else:
                            nc.vector.tensor_add(den_l, den_l, ptile[:, ds(0, QCH)])
                            nc.vector.tensor_add(den_r, den_r, ptile[:, ds(QCH, QCH)])
                        if diag and npairs > 1:
                            nc.vector.tensor_add(den, den_l, den_r)
                        ptiles[i] = ptile
                    j = i - LAG
                    if 0 <= j < npairs:
                        pt = ptiles.pop(j)
                        diagj = j == npairs - 1
                        _mm(nc, ps_o, v_sb[s][:, 2 * j, ds(h * 128, 128)], pt[:, ds(0, QCH)],
                            start=(j == 0), stop=False)
                        if diagj:
                            # denominator k-sum joins the O group (its row was
                            # pending-zero since the group start); the final O
                            # matmul below closes the group for all partitions
                            _mm(nc, ps_ob[0:1, ds(QCH, QCH)], ones_col, den, start=False, stop=False)
                            _mm(nc, ps_ob[:, ds(128, 128)], v_sb[s][:, 2 * j + 1, ds(h * 128, 128)],
                                pt[:, ds(QCH, 128)], start=False, stop=True)
                        else:
                            _mm(nc, ps_o, v_sb[s][:, 2 * j + 1, ds(h * 128, 128)], pt[:, ds(QCH, QCH)],
                                start=False, stop=False)
                # reciprocal of the k-sum (fp32), downcast to fp16, then an
                # outer-product matmul broadcasts it back to 128 partitions.
                recip = dp.tile([1, QCH], F32, tag="rcp", name="recip")
                nc.vector.reciprocal_approx_fast(out=recip, in_=ps_ob[0:1, ds(QCH, QCH)])
                recip16 = dp.tile([1, QCH], F16, tag="rcp16", name="recip16")
                nc.vector.tensor_copy(recip16, recip)
                ps_rbc = psR.tile([128, QCH], F32, tag="rbc", name="ps_rbc")
                _mm(nc, ps_rbc, ones_row, recip16, start=True, stop=True)
                # DVE can read only one PSUM operand: ACT evicts the raw O
                # tile to SBUF (freeing the psum bank early), DVE applies the
                # broadcast reciprocal.
                aoU = dp.tile([128, QCH], F16, tag="aoU", name="aoU")
                nc.scalar.copy(aoU, ps_o)
                nc.vector.tensor_mul(ao_t[s][h][:, qseg], aoU, ps_rbc)

    # out-projection partial for batch b
    def _stage_c(self, b, s, ao_t, wproj_sb, y, evp):
        nc, tc = self.nc, self.tc
        # last batch: fine-grained DMA per 512-block to shrink the drain tail
        fine = (b == B - 1)
        with tc.tile_pool(name=f"psY{b}", bufs=3, space="PSUM") as psY:
            for tt in range(TT):
                yv = evp.tile([128, C], F16, tag="yv", name="yv")
                for nck in range(C // 512):
                    ps_y = psY.tile([128, 512], F32, tag="y", name="ps_y")
                    for h in range(HPC):
                        _mm(nc, ps_y, ao_t[s][h][:, ds(tt * 128, 128)], wproj_sb[:, h, ds(nck * 512, 512)],
                            start=(h == 0), stop=(h == HPC - 1))
                    # alternate eviction engine: ACT alone can't keep pace
                    if nck % 2 == 0:
                        nc.scalar.copy(yv[:, ds(nck * 512, 512)], ps_y)
                    else:
                        nc.vector.tensor_copy(yv[:, ds(nck * 512, 512)], ps_y)
                    if fine:
                        nc.sync.dma_start(out=y[b, tt, :, ds(nck * 512, 512)], in_=yv[:, ds(nck * 512, 512)])
                if not fine:
                    # one batched DMA per token tile
                    nc.sync.dma_start(out=y[b, tt], in_=yv)


def prep_inputs(x, w_qkv, w_proj):
    """Host-side sharding: returns the per-core input maps."""
    x = np.asarray(x, dtype=np.float32)
    w_qkv = np.asarray(w_qkv, dtype=np.float32)
    w_proj = np.asarray(w_proj, dtype=np.float32)

    # x^T per batch: [B, C, T] -> tiled [B, KC, 128, T], fp16
    xt = np.ascontiguousarray(x.transpose(0, 2, 1)).astype(np.float16).reshape(B, KC, 128, T)

    # RoPE tables (mirror the fp32 reference computation)
    inv_freq = (1.0 / (10000.0 ** (np.arange(0, D, 2, dtype=np.float32) / D))).astype(np.float32)
    t = np.arange(T, dtype=np.float32)
    freqs = np.einsum("i,j->ij", t, inv_freq).astype(np.float32)  # [T, 64]
    emb = np.concatenate([freqs, freqs], axis=-1)  # [T, 128]
    cos_full = np.cos(emb).astype(np.float32)  # [T, 128]
    sin_full = np.sin(emb).astype(np.float32)
    sgn = np.where(np.arange(D) < D // 2, np.float32(-1.0), np.float32(1.0))
    cos_t = np.ascontiguousarray(cos_full[:, PERM].T).astype(np.float16)  # [128, T]
    sin_t = np.ascontiguousarray((sin_full * sgn)[:, PERM].T).astype(np.float16)

    # causal masks for the two k-tiles of a diagonal pair (q chunk = 256)
    kp = np.arange(128)[:, None]
    qf = np.arange(QCH)[None, :]
    qf128 = np.arange(128)[None, :]
    masks = np.concatenate(
        [(qf >= kp).astype(np.float16), (qf128 >= kp).astype(np.float16),
         np.zeros((128, 128), np.float16)], axis=1
    )  # [128, 512]: [tri256 | tri128 | unused]

    in_maps = []
    for g in range(NCORES):
        heads = [HPC * g + h for h in range(HPC)]
        # wqk: [C, 512] cols = [q_h0, q_h1, k_h0, k_h1], d-permuted
        cols = []
        for base in (0, C):  # q block, k block
            for hh in heads:
                cols.append(w_qkv[:, base + hh * 128 + PERM])
        wqk_g = np.ascontiguousarray(np.concatenate(cols, axis=1)).astype(np.float16).reshape(KC, 128, 512)
        wv_g = np.ascontiguousarray(
            np.concatenate([w_qkv[:, 2 * C + hh * 128:2 * C + (hh + 1) * 128] for hh in heads], axis=1)
        ).astype(np.float16).reshape(KC, 128, 256)
        wproj_g = np.ascontiguousarray(
            np.stack([w_proj[hh * 128:(hh + 1) * 128, :] for hh in heads])
        ).astype(np.float16)
        in_maps.append({
            "xt": xt,
            "wqk": wqk_g,
            "wv": wv_g,
            "wproj": wproj_g,
            "cos_t": cos_t,
            "sin_t": sin_t,
            "masks": masks,
            "ones_c": np.ones((128, 1), dtype=np.float16),
            "ones_r": np.ones((1, 128), dtype=np.float16),
        })
    return in_maps


_NC_CACHE = {}


def get_program():
    key = "v2"
    if key not in _NC_CACHE:
        _NC_CACHE[key] = build_program()
    return _NC_CACHE[key]


def kernel(x, w_qkv, w_proj, b_proj):
    from concourse import bass_utils

    nc = get_program()
    in_maps = prep_inputs(x, w_qkv, w_proj)
    res = bass_utils.run_bass_kernel_spmd(nc, in_maps, core_ids=list(range(NCORES)))
    acc = None
    for r in res.results:
        part = r["y"].astype(np.float32).reshape(B, T, C)
        acc = part if acc is None else acc + part
    return (acc + np.asarray(b_proj, dtype=np.float32)).astype(np.float32)


# revision 27
# speedup vs baseline: 1.0263x; 1.0263x over previous
"""Multi-head causal self-attention (RoPE) on 8 TRN2 NeuronCores.

Strategy (tensor-parallel over heads, per the sharding hint):
  - 16 heads / 8 cores -> 2 heads per core. Each core processes ALL 4
    batches for its 2 heads:
      qkv slice -> RoPE -> causal softmax(q k^T) v -> partial out-proj
    and writes a full-shape partial y (row-parallel w_proj). The host
    sums the 8 partials and adds b_proj.
  - All matmul operands are fp16 (PSUM accumulation stays fp32): the PE
    streams 1 row/cycle at any moving-dim size, DMA + SBUF traffic
    halve, and accuracy (~1e-3) is far inside the 2e-2 gate.
  - x is sent pre-transposed (x^T, [C, T] per batch) so the contraction
    dim C lands on SBUF partitions with no on-device transposes.
  - Attention runs in the "S^T" layout (k-tokens on partitions,
    q-tokens on the free dim), q-chunks of 256:
      S^T pair  = two matmuls (k-tiles 2p, 2p+1) into ONE psum bank
                  (start on the first zeroes the bank, stop on the 2nd)
      P^T pair  = exp(S^T * 1/sqrt(D))  (one ACT op per bank; no
                  max-subtraction: |scores| <~ 6 so exp is safe)
      denom     = DVE accumulates P columns, then ONE gpsimd
                  partition_all_reduce gives the broadcast k-sum
                  (no PE cycles and no separate broadcast)
      out^T    += v[ktile]^T P^T per k-tile, normalized at eviction by
                  reciprocal_approx_fast(denom).
    attn_out^T ([d, tok]) is directly the lhsT of the out-projection.
  - RoPE: the head dim d sits on partitions; rotate_half needs rows
    d <-> d+-64. We permute the d index on the host (within each head's
    128 columns of w_qkv + the cos/sin tables) so that rotation partners
    sit 16 apart inside the same 32-partition quadrant, which a single
    DVE stream_shuffle implements. Scores are invariant to the (shared)
    q/k permutation. The psum tile is first down-cast to fp16 by the
    ACT engine so all four RoPE DVE ops run on 16-bit data.
  - Per-batch q/k/v/attn-out stores are double-buffered (b%2) so batch
    b+1's projection overlaps batch b's attention/out-projection.
"""

import os
from contextlib import ExitStack

import numpy as np

import concourse.bacc as bacc
import concourse.bass as bass
import concourse.mybir as mybir
import concourse.tile as tile
from concourse.bass import ds, ts
from concourse.bass_isa import ReduceOp

B = 4
T = 2048
C = 2048
H = 16
D = 128
NCORES = 8
HPC = H // NCORES  # heads per core = 2
KC = C // 128  # 16 contraction tiles
TT = T // 128  # 16 token tiles
ACH = 256  # stage-A token chunk
NACH = T // ACH
QCH = 256  # stage-B q chunk
NQCH = T // QCH
INV_SQRT_D = float(1.0 / np.sqrt(np.float32(D)))

F32 = mybir.dt.float32
F16 = mybir.dt.float16

# d-permutation: quadrant s holds original d = s*16..s*16+15 (rows 0-15)
# and d+64 partners (rows 16-31); swap = stream_shuffle by +-16.
PERM = np.concatenate(
    [np.concatenate([np.arange(s * 16, s * 16 + 16), 64 + np.arange(s * 16, s * 16 + 16)]) for s in range(4)]
).astype(np.int64)
SWAP_MASK = [(i + 16) % 32 for i in range(32)]


def _mm(nc, out, lhsT, rhs, **kw):
    nc.tensor.matmul(out, lhsT, rhs, **kw)


def build_program():
    nc = bacc.Bacc("TRN2", target_bir_lowering=False, debug=False, num_devices=NCORES)

    xt = nc.dram_tensor("xt", [B, KC, 128, T], F16, kind="ExternalInput").ap()
    wqk = nc.dram_tensor("wqk", [KC, 128, 4 * 128], F16, kind="ExternalInput").ap()
    wv = nc.dram_tensor("wv", [KC, 128, 2 * 128], F16, kind="ExternalInput").ap()
    wproj = nc.dram_tensor("wproj", [HPC, 128, C], F16, kind="ExternalInput").ap()
    cos_in = nc.dram_tensor("cos_t", [128, T], F16, kind="ExternalInput").ap()
    sin_in = nc.dram_tensor("sin_t", [128, T], F16, kind="ExternalInput").ap()
    masks = nc.dram_tensor("masks", [128, 2 * QCH], F16, kind="ExternalInput").ap()
    ones_c = nc.dram_tensor("ones_c", [128, 1], F16, kind="ExternalInput").ap()
    ones_r = nc.dram_tensor("ones_r", [1, 128], F16, kind="ExternalInput").ap()
    y = nc.dram_tensor("y", [B, TT, 128, C], F16, kind="ExternalOutput").ap()

    with TileKernel(nc) as tk:
        tk.build(xt, wqk, wv, wproj, cos_in, sin_in, masks, ones_c, ones_r, y)
    nc.compile()
    return nc


class TileKernel:
    def __init__(self, nc):
        self.nc = nc
        self.stack = ExitStack()

    def __enter__(self):
        self.tc = self.stack.enter_context(tile.TileContext(self.nc))
        return self

    def __exit__(self, *exc):
        return self.stack.__exit__(*exc)

    def build(self, xt, wqk, wv, wproj, cos_in, sin_in, masks, ones_c, ones_r, y):
        nc, tc = self.nc, self.tc
        ctx = self.stack

        consts = ctx.enter_context(tc.tile_pool(name="consts", bufs=1))
        store = ctx.enter_context(tc.tile_pool(name="store", bufs=1))
        xtp = ctx.enter_context(tc.tile_pool(name="xtp", bufs=4))
        ropep = ctx.enter_context(tc.tile_pool(name="ropep", bufs=6))
        pp = ctx.enter_context(tc.tile_pool(name="pp", bufs=8))
        dp = ctx.enter_context(tc.tile_pool(name="dp", bufs=4))
        evp = ctx.enter_context(tc.tile_pool(name="evp", bufs=4))

        # prefetch the first two x chunks ahead of the bulk weight DMAs so
        # the first matmuls aren't stuck behind 3MB on the sync queue
        pf = {}
        for c in range(2):
            xt_pf = xtp.tile([128, KC, ACH], F16, tag="xt", name=f"xt_pf{c}")
            nc.sync.dma_start(out=xt_pf, in_=xt[0, :, :, ds(c * ACH, ACH)].transpose([1, 0, 2]))
            pf[c] = xt_pf

        # ---- persistent weights / tables ----
        wqk_sb = consts.tile([128, KC, 512], F16)
        wv_sb = consts.tile([128, KC, 256], F16)
        wproj_sb = consts.tile([128, HPC, C], F16)
        cos_sb = consts.tile([128, T], F16)
        sin_sb = consts.tile([128, T], F16)
        mask_sb = consts.tile([128, 2 * QCH], F16)
        ones_col = consts.tile([128, 1], F16)
        ones_row = consts.tile([1, 128], F16)
        nc.sync.dma_start(out=ones_col, in_=ones_c)
        nc.sync.dma_start(out=ones_row, in_=ones_r)
        # weights in kc-chunks so the first chunk's matmuls unblock before
        # the whole 3MB of weights has landed
        for k0 in range(0, KC, 4):
            nc.sync.dma_start(out=wqk_sb[:, ds(k0, 4), :], in_=wqk[ds(k0, 4)].transpose([1, 0, 2]))
            nc.sync.dma_start(out=wv_sb[:, ds(k0, 4), :], in_=wv[ds(k0, 4)].transpose([1, 0, 2]))
        nc.sync.dma_start(out=cos_sb, in_=cos_in)
        nc.sync.dma_start(out=sin_sb, in_=sin_in)

        # ---- per-batch stores, double-buffered over batches ----
        q_t = [[store.tile([128, T], F16, name=f"q_t{s}_{h}") for h in range(HPC)] for s in range(2)]
        k_t = [[store.tile([128, T], F16, name=f"k_t{s}_{h}") for h in range(HPC)] for s in range(2)]
        v_sb = [store.tile([128, TT, 256], F16, name=f"v_sb{s}") for s in range(2)]
        ao_t = [[store.tile([128, T], F16, name=f"ao_t{s}_{h}") for h in range(HPC)] for s in range(2)]

        for b in range(B):
            s = b % 2
            self._stage_a(b, s, xt, wqk_sb, wv_sb, cos_sb, sin_sb, q_t, k_t, v_sb, xtp, ropep, pf if b == 0 else None)
            if b == 0:
                # stage-B/C weights load behind batch-0 qkv so the first
                # matmuls aren't queued behind not-yet-needed DMAs
                nc.sync.dma_start(out=mask_sb, in_=masks)
                nc.sync.dma_start(out=wproj_sb, in_=wproj.transpose([1, 0, 2]))
            for h in range(HPC):
                self._stage_b(s, h, q_t, k_t, v_sb, ao_t, mask_sb, ones_col, ones_row, pp, dp)
            self._stage_c(b, s, ao_t, wproj_sb, y, evp)

    # qkv projection + RoPE for batch b
    def _stage_a(self, b, s, xt, wqk_sb, wv_sb, cos_sb, sin_sb, q_t, k_t, v_sb, xtp, ropep, pf=None):
        nc, tc = self.nc, self.tc
        # Two 256-wide accumulators share each PSUM bank; bufs=2 double-buffers
        # chunks so the PE never waits on the RoPE/v evictions.
        with tc.tile_pool(name=f"psA{b}", bufs=2, space="PSUM") as psA:
            for c in range(NACH):
                seg = ds(c * ACH, ACH)
                if pf and c in pf:
                    xt_all = pf.pop(c)
                else:
                    # one batched DMA per chunk: [KC, 128, ACH] -> [128, KC, ACH]
                    xt_all = xtp.tile([128, KC, ACH], F16, tag="xt", name=f"xt_{b}_{c}")
                    nc.sync.dma_start(out=xt_all, in_=xt[b, :, :, seg].transpose([1, 0, 2]))
                xt_tiles = [xt_all[:, kc, :] for kc in range(KC)]
                ps_b = [psA.tile([128, 2 * ACH], F32, tag=f"qkb{p}", name=f"psqkb{p}") for p in range(2)]
                ps_vb = psA.tile([128, 2 * 256], F32, tag="vb", name="psvb")
                ps_qk = [ps_b[m // 2][:, ds((m % 2) * ACH, ACH)] for m in range(4)]
                ps_v = [ps_vb[:, ds(t * 256, 256)] for t in range(ACH // 128)]
                # the two accumulators in one bank form a single group:
                # start zeroes the whole 2KB region, so only the first
                # matmul into a bank starts and only the last stops.
                for kc in range(KC):
                    for m in range(4):
                        _mm(nc, ps_qk[m], wqk_sb[:, kc, ds(m * 128, 128)], xt_tiles[kc],
                            start=(kc == 0 and m % 2 == 0), stop=(kc == KC - 1 and m % 2 == 1))
                    for t in range(ACH // 128):
                        _mm(nc, ps_v[t], xt_tiles[kc][:, ds(t * 128, 128)], wv_sb[:, kc, :],
                            start=(kc == 0 and t == 0), stop=(kc == KC - 1 and t == 1))
                # RoPE eviction: m -> (q/k, head). ACT downcasts the psum
                # tile to fp16 first so the DVE ops run on 16-bit data.
                for m in range(4):
                    h = m % 2
                    dst = (q_t if m < 2 else k_t)[s][h]
                    qf = ropep.tile([128, ACH], F16, tag="qf", name="qf")
                    sw = ropep.tile([128, ACH], F16, tag="sw", name="sw")
                    t1 = ropep.tile([128, ACH], F16, tag="t1", name="t1")
                    nc.scalar.copy(qf, ps_qk[m])
                    nc.vector.stream_shuffle(sw, qf, mask=SWAP_MASK)
                    nc.vector.tensor_mul(t1, qf, cos_sb[:, seg])
                    nc.vector.tensor_mul(sw, sw, sin_sb[:, seg])
                    nc.vector.tensor_add(dst[:, seg], t1, sw)
                for t in range(ACH // 128):
                    nc.scalar.copy(v_sb[s][:, c * (ACH // 128) + t, :], ps_v[t])

    # causal attention for head h (current batch): fills ao_t[s][h]
    def _stage_b(self, s, h, q_t, k_t, v_sb, ao_t, mask_sb, ones_col, ones_row, pp, dp):
        nc, tc = self.nc, self.tc
        with (
            tc.tile_pool(name=f"psS{h}", bufs=3, space="PSUM") as psS,
            tc.tile_pool(name=f"psO{h}", bufs=3, space="PSUM") as psO,
            tc.tile_pool(name=f"psR{h}", bufs=2, space="PSUM") as psR,
        ):
            for jc in range(NQCH):
                qseg = ds(jc * QCH, QCH)
                npairs = jc + 1  # k-tile pairs 2p, 2p+1 with 2p+1 <= 2jc+1
                # O accumulator shares its psum bank with the denominator
                # row: the den-sum matmul is the group's closing member.
                ps_ob = psO.tile([128, 2 * QCH], F32, tag="o", name="ps_ob")
                ps_o = ps_ob[:, ds(0, QCH)]
                # two independent denominator accumulators (left/right pair
                # halves) halve the serial DVE chain that paces the pipeline
                den_l = dp.tile([128, QCH], F16, tag="denl", name="den_l")
                den_r = dp.tile([128, QCH], F16, tag="denr", name="den_r")
                den = dp.tile([128, QCH], F16, tag="den", name="den")
                # software pipeline: the O matmuls consume ptile LAG pairs
                # behind the S-matmul/exp/mask producers so the PE never
                # waits on ACT/DVE.
                LAG = 2
                ptiles = {}
                for i in range(npairs + LAG):
                    if i < npairs:
                        ps_s = psS.tile([128, 2 * QCH], F32, tag="s", name="ps_s")
                        _mm(nc, ps_s[:, ds(0, QCH)], k_t[s][h][:, ds(2 * i * 128, 128)], q_t[s][h][:, qseg],
                            start=True, stop=False)
                        _mm(nc, ps_s[:, ds(QCH, QCH)], k_t[s][h][:, ds((2 * i + 1) * 128, 128)], q_t[s][h][:, qseg],
                            start=False, stop=True)
                        ptile = pp.tile([128, 2 * QCH], F16, tag="pt", name="ptile")
                        nc.scalar.activation(ptile, ps_s, mybir.ActivationFunctionType.Exp, scale=INV_SQRT_D)
                        if i == npairs - 1:  # diagonal pair
                            nc.vector.tensor_mul(ptile, ptile, mask_sb)
                        if i == 0:
                            nc.vector.tensor_copy(den_l, ptile[:, ds(0, QCH)])
                            nc.vector.tensor_copy(den_r, ptile[:, ds(QCH, QCH)])
                        else:
                            nc.vector.tensor_add(den_l, den_l, ptile[:, ds(0, QCH)])
                            nc.vector.tensor_add(den_r, den_r, ptile[:, ds(QCH, QCH)])
                        if i == npairs - 1:
                            nc.vector.tensor_add(den, den_l, den_r)
                        ptiles[i] = ptile
                    j = i - LAG
                    if 0 <= j < npairs:
                        pt = ptiles.pop(j)
                        _mm(nc, ps_o, v_sb[s][:, 2 * j, ds(h * 128, 128)], pt[:, ds(0, QCH)],
                            start=(j == 0), stop=False)
                        if j == npairs - 1:
                            # denominator k-sum joins the O group (its row was
                            # pending-zero since the group start); the final O
                            # matmul below closes the group for all partitions
                            _mm(nc, ps_ob[0:1, ds(QCH, QCH)], ones_col, den, start=False, stop=False)
                        _mm(nc, ps_o, v_sb[s][:, 2 * j + 1, ds(h * 128, 128)], pt[:, ds(QCH, QCH)],
                            start=False, stop=(j == npairs - 1))
                # reciprocal of the k-sum (fp32), downcast to fp16, then an
                # outer-product matmul broadcasts it back to 128 partitions.
                recip = dp.tile([1, QCH], F32, tag="rcp", name="recip")
                nc.vector.reciprocal_approx_fast(out=recip, in_=ps_ob[0:1, ds(QCH, QCH)])
                recip16 = dp.tile([1, QCH], F16, tag="rcp16", name="recip16")
                nc.vector.tensor_copy(recip16, recip)
                ps_rbc = psR.tile([128, QCH], F32, tag="rbc", name="ps_rbc")
                _mm(nc, ps_rbc, ones_row, recip16, start=True, stop=True)
                # DVE can read only one PSUM operand: ACT evicts the raw O
                # tile to SBUF (freeing the psum bank early), DVE applies the
                # broadcast reciprocal.
                aoU = dp.tile([128, QCH], F16, tag="aoU", name="aoU")
                nc.scalar.copy(aoU, ps_o)
                nc.vector.tensor_mul(ao_t[s][h][:, qseg], aoU, ps_rbc)

    # out-projection partial for batch b
    def _stage_c(self, b, s, ao_t, wproj_sb, y, evp):
        nc, tc = self.nc, self.tc
        with tc.tile_pool(name=f"psY{b}", bufs=3, space="PSUM") as psY:
            for tt in range(TT):
                yv = evp.tile([128, C], F16, tag="yv", name="yv")
                for nck in range(C // 512):
                    ps_y = psY.tile([128, 512], F32, tag="y", name="ps_y")
                    for h in range(HPC):
                        _mm(nc, ps_y, ao_t[s][h][:, ds(tt * 128, 128)], wproj_sb[:, h, ds(nck * 512, 512)],
                            start=(h == 0), stop=(h == HPC - 1))
                    # alternate eviction engine: ACT alone can't keep pace
                    if nck % 2 == 0:
                        nc.scalar.copy(yv[:, ds(nck * 512, 512)], ps_y)
                    else:
                        nc.vector.tensor_copy(yv[:, ds(nck * 512, 512)], ps_y)
                # one batched DMA per token tile
                nc.sync.dma_start(out=y[b, tt], in_=yv)


def prep_inputs(x, w_qkv, w_proj):
    """Host-side sharding: returns the per-core input maps."""
    x = np.asarray(x, dtype=np.float32)
    w_qkv = np.asarray(w_qkv, dtype=np.float32)
    w_proj = np.asarray(w_proj, dtype=np.float32)

    # x^T per batch: [B, C, T] -> tiled [B, KC, 128, T], fp16
    xt = np.ascontiguousarray(x.transpose(0, 2, 1)).astype(np.float16).reshape(B, KC, 128, T)

    # RoPE tables (mirror the fp32 reference computation)
    inv_freq = (1.0 / (10000.0 ** (np.arange(0, D, 2, dtype=np.float32) / D))).astype(np.float32)
    t = np.arange(T, dtype=np.float32)
    freqs = np.einsum("i,j->ij", t, inv_freq).astype(np.float32)  # [T, 64]
    emb = np.concatenate([freqs, freqs], axis=-1)  # [T, 128]
    cos_full = np.cos(emb).astype(np.float32)  # [T, 128]
    sin_full = np.sin(emb).astype(np.float32)
    sgn = np.where(np.arange(D) < D // 2, np.float32(-1.0), np.float32(1.0))
    cos_t = np.ascontiguousarray(cos_full[:, PERM].T).astype(np.float16)  # [128, T]
    sin_t = np.ascontiguousarray((sin_full * sgn)[:, PERM].T).astype(np.float16)

    # causal masks for the two k-tiles of a diagonal pair (q chunk = 256)
    kp = np.arange(128)[:, None]
    qf = np.arange(QCH)[None, :]
    masks = np.concatenate(
        [(qf >= kp).astype(np.float16), (qf >= 128 + kp).astype(np.float16)], axis=1
    )  # [128, 512]

    in_maps = []
    for g in range(NCORES):
        heads = [HPC * g + h for h in range(HPC)]
        # wqk: [C, 512] cols = [q_h0, q_h1, k_h0, k_h1], d-permuted
        cols = []
        for base in (0, C):  # q block, k block
            for hh in heads:
                cols.append(w_qkv[:, base + hh * 128 + PERM])
        wqk_g = np.ascontiguousarray(np.concatenate(cols, axis=1)).astype(np.float16).reshape(KC, 128, 512)
        wv_g = np.ascontiguousarray(
            np.concatenate([w_qkv[:, 2 * C + hh * 128:2 * C + (hh + 1) * 128] for hh in heads], axis=1)
        ).astype(np.float16).reshape(KC, 128, 256)
        wproj_g = np.ascontiguousarray(
            np.stack([w_proj[hh * 128:(hh + 1) * 128, :] for hh in heads])
        ).astype(np.float16)
        in_maps.append({
            "xt": xt,
            "wqk": wqk_g,
            "wv": wv_g,
            "wproj": wproj_g,
            "cos_t": cos_t,
            "sin_t": sin_t,
            "masks": masks,
            "ones_c": np.ones((128, 1), dtype=np.float16),
            "ones_r": np.ones((1, 128), dtype=np.float16),
        })
    return in_maps


_NC_CACHE = {}


def get_program():
    key = "v2"
    if key not in _NC_CACHE:
        _NC_CACHE[key] = build_program()
    return _NC_CACHE[key]


def kernel(x, w_qkv, w_proj, b_proj):
    from concourse import bass_utils

    nc = get_program()
    in_maps = prep_inputs(x, w_qkv, w_proj)
    res = bass_utils.run_bass_kernel_spmd(nc, in_maps, core_ids=list(range(NCORES)))
    acc = None
    for r in res.results:
        part = r["y"].astype(np.float32).reshape(B, T, C)
        acc = part if acc is None else acc + part
    return (acc + np.asarray(b_proj, dtype=np.float32)).astype(np.float32)


# revision 28
# speedup vs baseline: 1.0293x; 1.0029x over previous
"""Multi-head causal self-attention (RoPE) on 8 TRN2 NeuronCores.

Strategy (tensor-parallel over heads, per the sharding hint):
  - 16 heads / 8 cores -> 2 heads per core. Each core processes ALL 4
    batches for its 2 heads:
      qkv slice -> RoPE -> causal softmax(q k^T) v -> partial out-proj
    and writes a full-shape partial y (row-parallel w_proj). The host
    sums the 8 partials and adds b_proj.
  - All matmul operands are fp16 (PSUM accumulation stays fp32): the PE
    streams 1 row/cycle at any moving-dim size, DMA + SBUF traffic
    halve, and accuracy (~1e-3) is far inside the 2e-2 gate.
  - x is sent pre-transposed (x^T, [C, T] per batch) so the contraction
    dim C lands on SBUF partitions with no on-device transposes.
  - Attention runs in the "S^T" layout (k-tokens on partitions,
    q-tokens on the free dim), q-chunks of 256:
      S^T pair  = two matmuls (k-tiles 2p, 2p+1) into ONE psum bank
                  (start on the first zeroes the bank, stop on the 2nd)
      P^T pair  = exp(S^T * 1/sqrt(D))  (one ACT op per bank; no
                  max-subtraction: |scores| <~ 6 so exp is safe)
      denom     = DVE accumulates P columns into two chains (left/
                  right halves), a 1-col ones matmul in the O bank's
                  spare half does the k-sum (joining the O group), DVE
                  takes the fast reciprocal, and an outer-product
                  matmul broadcasts it to 128 partitions; ACT evicts
                  the raw O tile and DVE applies the reciprocal (DVE
                  reads at most one PSUM operand per op).
    attn_out^T ([d, tok]) is directly the lhsT of the out-projection.
  - RoPE: the head dim d sits on partitions; rotate_half needs rows
    d <-> d+-64. We permute the d index on the host (within each head's
    128 columns of w_qkv + the cos/sin tables) so that rotation partners
    sit 16 apart inside the same 32-partition quadrant, which a single
    DVE stream_shuffle implements. Scores are invariant to the (shared)
    q/k permutation. The psum tile is first down-cast to fp16 by the
    ACT engine so all four RoPE DVE ops run on 16-bit data.
  - Per-batch q/k/v/attn-out stores are double-buffered (b%2) so batch
    b+1's projection overlaps batch b's attention/out-projection.
"""

import os
from contextlib import ExitStack

import numpy as np

import concourse.bacc as bacc
import concourse.mybir as mybir
import concourse.tile as tile
from concourse.bass import ds

B = 4
T = 2048
C = 2048
H = 16
D = 128
NCORES = 8
HPC = H // NCORES  # heads per core = 2
KC = C // 128  # 16 contraction tiles
TT = T // 128  # 16 token tiles
ACH = 256  # stage-A token chunk
NACH = T // ACH
QCH = 256  # stage-B q chunk
NQCH = T // QCH
INV_SQRT_D = float(1.0 / np.sqrt(np.float32(D)))

F32 = mybir.dt.float32
F16 = mybir.dt.float16

# d-permutation: quadrant s holds original d = s*16..s*16+15 (rows 0-15)
# and d+64 partners (rows 16-31); swap = stream_shuffle by +-16.
PERM = np.concatenate(
    [np.concatenate([np.arange(s * 16, s * 16 + 16), 64 + np.arange(s * 16, s * 16 + 16)]) for s in range(4)]
).astype(np.int64)
SWAP_MASK = [(i + 16) % 32 for i in range(32)]


def _mm(nc, out, lhsT, rhs, **kw):
    nc.tensor.matmul(out, lhsT, rhs, **kw)


def build_program():
    nc = bacc.Bacc("TRN2", target_bir_lowering=False, debug=False, num_devices=NCORES)

    xt = nc.dram_tensor("xt", [B, KC, 128, T], F16, kind="ExternalInput").ap()
    wqk = nc.dram_tensor("wqk", [KC, 128, 4 * 128], F16, kind="ExternalInput").ap()
    wv = nc.dram_tensor("wv", [KC, 128, 2 * 128], F16, kind="ExternalInput").ap()
    wproj = nc.dram_tensor("wproj", [HPC, 128, C], F16, kind="ExternalInput").ap()
    cos_in = nc.dram_tensor("cos_t", [128, T], F16, kind="ExternalInput").ap()
    sin_in = nc.dram_tensor("sin_t", [128, T], F16, kind="ExternalInput").ap()
    masks = nc.dram_tensor("masks", [128, 2 * QCH], F16, kind="ExternalInput").ap()
    ones_c = nc.dram_tensor("ones_c", [128, 1], F16, kind="ExternalInput").ap()
    ones_r = nc.dram_tensor("ones_r", [1, 128], F16, kind="ExternalInput").ap()
    y = nc.dram_tensor("y", [B, TT, 128, C], F16, kind="ExternalOutput").ap()

    with TileKernel(nc) as tk:
        tk.build(xt, wqk, wv, wproj, cos_in, sin_in, masks, ones_c, ones_r, y)
    nc.compile()
    return nc


class TileKernel:
    def __init__(self, nc):
        self.nc = nc
        self.stack = ExitStack()

    def __enter__(self):
        self.tc = self.stack.enter_context(tile.TileContext(self.nc))
        return self

    def __exit__(self, *exc):
        return self.stack.__exit__(*exc)

    def build(self, xt, wqk, wv, wproj, cos_in, sin_in, masks, ones_c, ones_r, y):
        nc, tc = self.nc, self.tc
        ctx = self.stack

        consts = ctx.enter_context(tc.tile_pool(name="consts", bufs=1))
        store = ctx.enter_context(tc.tile_pool(name="store", bufs=1))
        xtp = ctx.enter_context(tc.tile_pool(name="xtp", bufs=4))
        ropep = ctx.enter_context(tc.tile_pool(name="ropep", bufs=6))
        pp = ctx.enter_context(tc.tile_pool(name="pp", bufs=8))
        dp = ctx.enter_context(tc.tile_pool(name="dp", bufs=4))
        evp = ctx.enter_context(tc.tile_pool(name="evp", bufs=4))

        # prefetch the first two x chunks ahead of the bulk weight DMAs so
        # the first matmuls aren't stuck behind 3MB on the sync queue
        pf = {}
        for c in range(2):
            xt_pf = xtp.tile([128, KC, ACH], F16, tag="xt", name=f"xt_pf{c}")
            nc.sync.dma_start(out=xt_pf, in_=xt[0, :, :, ds(c * ACH, ACH)].transpose([1, 0, 2]))
            pf[c] = xt_pf

        # ---- persistent weights / tables ----
        wqk_sb = consts.tile([128, KC, 512], F16)
        wv_sb = consts.tile([128, KC, 256], F16)
        wproj_sb = consts.tile([128, HPC, C], F16)
        cos_sb = consts.tile([128, T], F16)
        sin_sb = consts.tile([128, T], F16)
        mask_sb = consts.tile([128, 2 * QCH], F16)
        ones_col = consts.tile([128, 1], F16)
        ones_row = consts.tile([1, 128], F16)
        nc.sync.dma_start(out=ones_col, in_=ones_c)
        nc.sync.dma_start(out=ones_row, in_=ones_r)
        # weights in kc-chunks so the first chunk's matmuls unblock before
        # the whole 3MB of weights has landed
        for k0 in range(0, KC, 4):
            nc.sync.dma_start(out=wqk_sb[:, ds(k0, 4), :], in_=wqk[ds(k0, 4)].transpose([1, 0, 2]))
            nc.sync.dma_start(out=wv_sb[:, ds(k0, 4), :], in_=wv[ds(k0, 4)].transpose([1, 0, 2]))
        nc.sync.dma_start(out=cos_sb, in_=cos_in)
        nc.sync.dma_start(out=sin_sb, in_=sin_in)

        # ---- per-batch stores, double-buffered over batches ----
        q_t = [[store.tile([128, T], F16, name=f"q_t{s}_{h}") for h in range(HPC)] for s in range(2)]
        k_t = [[store.tile([128, T], F16, name=f"k_t{s}_{h}") for h in range(HPC)] for s in range(2)]
        v_sb = [store.tile([128, TT, 256], F16, name=f"v_sb{s}") for s in range(2)]
        ao_t = [[store.tile([128, T], F16, name=f"ao_t{s}_{h}") for h in range(HPC)] for s in range(2)]

        for b in range(B):
            s = b % 2
            self._stage_a(b, s, xt, wqk_sb, wv_sb, cos_sb, sin_sb, q_t, k_t, v_sb, xtp, ropep, pf if b == 0 else None)
            if b == 0:
                # stage-B/C weights load behind batch-0 qkv so the first
                # matmuls aren't queued behind not-yet-needed DMAs
                nc.sync.dma_start(out=mask_sb, in_=masks)
                nc.sync.dma_start(out=wproj_sb, in_=wproj.transpose([1, 0, 2]))
            for h in range(HPC):
                self._stage_b(s, h, q_t, k_t, v_sb, ao_t, mask_sb, ones_col, ones_row, pp, dp)
            self._stage_c(b, s, ao_t, wproj_sb, y, evp)

    # qkv projection + RoPE for batch b
    def _stage_a(self, b, s, xt, wqk_sb, wv_sb, cos_sb, sin_sb, q_t, k_t, v_sb, xtp, ropep, pf=None):
        nc, tc = self.nc, self.tc
        # Two 256-wide accumulators share each PSUM bank; bufs=2 double-buffers
        # chunks so the PE never waits on the RoPE/v evictions.
        with tc.tile_pool(name=f"psA{b}", bufs=2, space="PSUM") as psA:
            for c in range(NACH):
                seg = ds(c * ACH, ACH)
                if pf and c in pf:
                    xt_all = pf.pop(c)
                else:
                    # one batched DMA per chunk: [KC, 128, ACH] -> [128, KC, ACH]
                    xt_all = xtp.tile([128, KC, ACH], F16, tag="xt", name=f"xt_{b}_{c}")
                    nc.sync.dma_start(out=xt_all, in_=xt[b, :, :, seg].transpose([1, 0, 2]))
                xt_tiles = [xt_all[:, kc, :] for kc in range(KC)]
                ps_b = [psA.tile([128, 2 * ACH], F32, tag=f"qkb{p}", name=f"psqkb{p}") for p in range(2)]
                ps_vb = psA.tile([128, 2 * 256], F32, tag="vb", name="psvb")
                ps_qk = [ps_b[m // 2][:, ds((m % 2) * ACH, ACH)] for m in range(4)]
                ps_v = [ps_vb[:, ds(t * 256, 256)] for t in range(ACH // 128)]
                # the two accumulators in one bank form a single group:
                # start zeroes the whole 2KB region, so only the first
                # matmul into a bank starts and only the last stops.
                for kc in range(KC):
                    for m in range(4):
                        _mm(nc, ps_qk[m], wqk_sb[:, kc, ds(m * 128, 128)], xt_tiles[kc],
                            start=(kc == 0 and m % 2 == 0), stop=(kc == KC - 1 and m % 2 == 1))
                    for t in range(ACH // 128):
                        _mm(nc, ps_v[t], xt_tiles[kc][:, ds(t * 128, 128)], wv_sb[:, kc, :],
                            start=(kc == 0 and t == 0), stop=(kc == KC - 1 and t == 1))
                # RoPE eviction: m -> (q/k, head). ACT downcasts the psum
                # tile to fp16 first so the DVE ops run on 16-bit data.
                for m in range(4):
                    h = m % 2
                    dst = (q_t if m < 2 else k_t)[s][h]
                    qf = ropep.tile([128, ACH], F16, tag="qf", name="qf")
                    sw = ropep.tile([128, ACH], F16, tag="sw", name="sw")
                    t1 = ropep.tile([128, ACH], F16, tag="t1", name="t1")
                    nc.scalar.copy(qf, ps_qk[m])
                    nc.vector.stream_shuffle(sw, qf, mask=SWAP_MASK)
                    nc.vector.tensor_mul(t1, qf, cos_sb[:, seg])
                    nc.vector.tensor_mul(sw, sw, sin_sb[:, seg])
                    nc.vector.tensor_add(dst[:, seg], t1, sw)
                for t in range(ACH // 128):
                    nc.scalar.copy(v_sb[s][:, c * (ACH // 128) + t, :], ps_v[t])

    # causal attention for head h (current batch): fills ao_t[s][h]
    def _stage_b(self, s, h, q_t, k_t, v_sb, ao_t, mask_sb, ones_col, ones_row, pp, dp):
        nc, tc = self.nc, self.tc
        with (
            tc.tile_pool(name=f"psS{h}", bufs=3, space="PSUM") as psS,
            tc.tile_pool(name=f"psO{h}", bufs=3, space="PSUM") as psO,
            tc.tile_pool(name=f"psR{h}", bufs=2, space="PSUM") as psR,
        ):
            for jc in range(NQCH):
                qseg = ds(jc * QCH, QCH)
                npairs = jc + 1  # k-tile pairs 2p, 2p+1 with 2p+1 <= 2jc+1
                # O accumulator shares its psum bank with the denominator
                # row: the den-sum matmul is the group's closing member.
                ps_ob = psO.tile([128, 2 * QCH], F32, tag="o", name="ps_ob")
                ps_o = ps_ob[:, ds(0, QCH)]
                # two independent denominator accumulators (left/right pair
                # halves) halve the serial DVE chain that paces the pipeline
                den_l = dp.tile([128, QCH], F16, tag="denl", name="den_l")
                den_r = dp.tile([128, QCH], F16, tag="denr", name="den_r")
                den = dp.tile([128, QCH], F16, tag="den", name="den")
                # software pipeline: the O matmuls consume ptile LAG pairs
                # behind the S-matmul/exp/mask producers so the PE never
                # waits on ACT/DVE.
                LAG = 2
                ptiles = {}
                for i in range(npairs + LAG):
                    if i < npairs:
                        ps_s = psS.tile([128, 2 * QCH], F32, tag="s", name="ps_s")
                        _mm(nc, ps_s[:, ds(0, QCH)], k_t[s][h][:, ds(2 * i * 128, 128)], q_t[s][h][:, qseg],
                            start=True, stop=False)
                        _mm(nc, ps_s[:, ds(QCH, QCH)], k_t[s][h][:, ds((2 * i + 1) * 128, 128)], q_t[s][h][:, qseg],
                            start=False, stop=True)
                        ptile = pp.tile([128, 2 * QCH], F16, tag="pt", name="ptile")
                        nc.scalar.activation(ptile, ps_s, mybir.ActivationFunctionType.Exp, scale=INV_SQRT_D)
                        if i == npairs - 1:  # diagonal pair
                            nc.vector.tensor_mul(ptile, ptile, mask_sb)
                        if i == 0:
                            nc.vector.tensor_copy(den_l, ptile[:, ds(0, QCH)])
                            nc.vector.tensor_copy(den_r, ptile[:, ds(QCH, QCH)])
                        else:
                            nc.vector.tensor_add(den_l, den_l, ptile[:, ds(0, QCH)])
                            nc.vector.tensor_add(den_r, den_r, ptile[:, ds(QCH, QCH)])
                        if i == npairs - 1:
                            nc.vector.tensor_add(den, den_l, den_r)
                        ptiles[i] = ptile
                    j = i - LAG
                    if 0 <= j < npairs:
                        pt = ptiles.pop(j)
                        _mm(nc, ps_o, v_sb[s][:, 2 * j, ds(h * 128, 128)], pt[:, ds(0, QCH)],
                            start=(j == 0), stop=False)
                        if j == npairs - 1:
                            # denominator k-sum joins the O group (its row was
                            # pending-zero since the group start); the final O
                            # matmul below closes the group for all partitions
                            _mm(nc, ps_ob[0:1, ds(QCH, QCH)], ones_col, den, start=False, stop=False)
                        _mm(nc, ps_o, v_sb[s][:, 2 * j + 1, ds(h * 128, 128)], pt[:, ds(QCH, QCH)],
                            start=False, stop=(j == npairs - 1))
                # reciprocal of the k-sum (fp32), downcast to fp16, then an
                # outer-product matmul broadcasts it back to 128 partitions.
                recip = dp.tile([1, QCH], F32, tag="rcp", name="recip")
                nc.vector.reciprocal_approx_fast(out=recip, in_=ps_ob[0:1, ds(QCH, QCH)])
                recip16 = dp.tile([1, QCH], F16, tag="rcp16", name="recip16")
                nc.vector.tensor_copy(recip16, recip)
                ps_rbc = psR.tile([128, QCH], F32, tag="rbc", name="ps_rbc")
                _mm(nc, ps_rbc, ones_row, recip16, start=True, stop=True)
                # DVE can read only one PSUM operand: ACT evicts the raw O
                # tile to SBUF (freeing the psum bank early), DVE applies the
                # broadcast reciprocal.
                aoU = dp.tile([128, QCH], F16, tag="aoU", name="aoU")
                nc.scalar.copy(aoU, ps_o)
                nc.vector.tensor_mul(ao_t[s][h][:, qseg], aoU, ps_rbc)

    # out-projection partial for batch b
    def _stage_c(self, b, s, ao_t, wproj_sb, y, evp):
        nc, tc = self.nc, self.tc
        with tc.tile_pool(name=f"psY{b}", bufs=3, space="PSUM") as psY:
            for tt in range(TT):
                yv = evp.tile([128, C], F16, tag="yv", name="yv")
                for nck in range(C // 512):
                    ps_y = psY.tile([128, 512], F32, tag="y", name="ps_y")
                    for h in range(HPC):
                        _mm(nc, ps_y, ao_t[s][h][:, ds(tt * 128, 128)], wproj_sb[:, h, ds(nck * 512, 512)],
                            start=(h == 0), stop=(h == HPC - 1))
                    # alternate eviction engine: ACT alone can't keep pace
                    if nck % 2 == 0:
                        nc.scalar.copy(yv[:, ds(nck * 512, 512)], ps_y)
                    else:
                        nc.vector.tensor_copy(yv[:, ds(nck * 512, 512)], ps_y)
                # one batched DMA per token tile
                nc.sync.dma_start(out=y[b, tt], in_=yv)


def prep_inputs(x, w_qkv, w_proj):
    """Host-side sharding: returns the per-core input maps."""
    x = np.asarray(x, dtype=np.float32)
    w_qkv = np.asarray(w_qkv, dtype=np.float32)
    w_proj = np.asarray(w_proj, dtype=np.float32)

    # x^T per batch: [B, C, T] -> tiled [B, KC, 128, T], fp16
    xt = np.ascontiguousarray(x.transpose(0, 2, 1)).astype(np.float16).reshape(B, KC, 128, T)

    # RoPE tables (mirror the fp32 reference computation)
    inv_freq = (1.0 / (10000.0 ** (np.arange(0, D, 2, dtype=np.float32) / D))).astype(np.float32)
    t = np.arange(T, dtype=np.float32)
    freqs = np.einsum("i,j->ij", t, inv_freq).astype(np.float32)  # [T, 64]
    emb = np.concatenate([freqs, freqs], axis=-1)  # [T, 128]
    cos_full = np.cos(emb).astype(np.float32)  # [T, 128]
    sin_full = np.sin(emb).astype(np.float32)
    sgn = np.where(np.arange(D) < D // 2, np.float32(-1.0), np.float32(1.0))
    cos_t = np.ascontiguousarray(cos_full[:, PERM].T).astype(np.float16)  # [128, T]
    sin_t = np.ascontiguousarray((sin_full * sgn)[:, PERM].T).astype(np.float16)

    # causal masks for the two k-tiles of a diagonal pair (q chunk = 256)
    kp = np.arange(128)[:, None]
    qf = np.arange(QCH)[None, :]
    masks = np.concatenate(
        [(qf >= kp).astype(np.float16), (qf >= 128 + kp).astype(np.float16)], axis=1
    )  # [128, 512]

    in_maps = []
    for g in range(NCORES):
        heads = [HPC * g + h for h in range(HPC)]
        # wqk: [C, 512] cols = [q_h0, q_h1, k_h0, k_h1], d-permuted
        cols = []
        for base in (0, C):  # q block, k block
            for hh in heads:
                cols.append(w_qkv[:, base + hh * 128 + PERM])
        wqk_g = np.ascontiguousarray(np.concatenate(cols, axis=1)).astype(np.float16).reshape(KC, 128, 512)
        wv_g = np.ascontiguousarray(
            np.concatenate([w_qkv[:, 2 * C + hh * 128:2 * C + (hh + 1) * 128] for hh in heads], axis=1)
        ).astype(np.float16).reshape(KC, 128, 256)
        wproj_g = np.ascontiguousarray(
            np.stack([w_proj[hh * 128:(hh + 1) * 128, :] for hh in heads])
        ).astype(np.float16)
        in_maps.append({
            "xt": xt,
            "wqk": wqk_g,
            "wv": wv_g,
            "wproj": wproj_g,
            "cos_t": cos_t,
            "sin_t": sin_t,
            "masks": masks,
            "ones_c": np.ones((128, 1), dtype=np.float16),
            "ones_r": np.ones((1, 128), dtype=np.float16),
        })
    return in_maps


_NC_CACHE = {}


def get_program():
    key = "v2"
    if key not in _NC_CACHE:
        _NC_CACHE[key] = build_program()
    return _NC_CACHE[key]


def kernel(x, w_qkv, w_proj, b_proj):
    from concourse import bass_utils

    nc = get_program()
    in_maps = prep_inputs(x, w_qkv, w_proj)
    res = bass_utils.run_bass_kernel_spmd(nc, in_maps, core_ids=list(range(NCORES)))
    acc = None
    for r in res.results:
        part = r["y"].astype(np.float32).reshape(B, T, C)
        acc = part if acc is None else acc + part
    return (acc + np.asarray(b_proj, dtype=np.float32)).astype(np.float32)
